# revision 49
# baseline (speedup 1.0000x reference)
"""Trainium2 Bass kernel for causal multi-head attention (b=2, n=2048, d=1024, h=16).

Sharding: 8 cores; core c handles batch (c // 4) and the 4 heads
[4*(c%4), 4*(c%4)+4).  Each core computes its heads' attention plus its
partial output projection y_part = O_heads @ Wo[:, cols].T ; the host sums
the 4 per-batch partials (bf16 on the wire) and adds bo + bv @ Wo.T (the
V-bias contribution commutes with the softmax average, so it never goes on
device).

All matmuls run in bf16 with fp32 PSUM accumulation.  The PE is the
bottleneck engine (~119us of streaming at 2.4 GHz), and TRN2 halves the PE
clock for 3us after any idle gap, so the emission schedule is built around
keeping the PE saturated:

  - engine assignment: ACT does (almost) nothing but the softmax exp;
    Q/K/V projection drains run on the DVE as tensor_scalar ops (the
    per-partition scale/bias replaces ACT's activation bias/scale);
    normalize finish-muls run on GpSimd (SBUF-only engine); y drains are
    split ACT/DVE.
  - projection and output-projection matmul "units" (one PSUM accumulation
    group each) are queued as fillers and emitted BETWEEN attention pairs,
    absorbing the ACT-vs-PE drift (exp takes ~1.14us per 1024-col pair vs
    ~0.85us of PE work per pair).  Projection units flush before the
    attend phase that needs them; outproj units carry across boundaries.
  - head: the first wq/x pieces are split small, and the bulk transfers
    (x blocks 2-3, Wo, mask) are gated behind an SBUF->SBUF guard DMA
    sourced from x block 0, so the critical first ~4MB gets the full HBM
    bandwidth while warm-up junk matmuls hold the PE clock at speed.
  - tail: the last q-tile processes heads in order (2,3,0,1), holds back
    outproj(1)/outproj(2) units as late fillers, and normalizes its final
    two heads with a PE ones-matmul broadcast of (1/)l instead of the DMA
    bounce, with a couple of junk matmuls keeping the clock hot through
    the dependency chain.

Causal masking multiplies only the 128-col diagonal band of each PT chunk
with a constant 0/1 mask slice; fully-masked columns are never exp'd or
streamed.  The softmax denominator l rides the PV matmul as a ones column
of V_aug (partition 64 for even heads / 0 for odd); 1/l is broadcast
across partitions with an SBUF->DRAM->SBUF DMA bounce, finished two heads
later so the engines never stall on the round trip.
"""

import numpy as np

import concourse.bass as bass
import concourse.mybir as mybir
import concourse.tile as tile
from concourse import bacc
from concourse.bass_utils import run_bass_kernel_spmd

D = 1024          # d_model
N = 2048          # sequence length
B = 2             # batch
H_TOT = 16        # total heads
HD = 64           # head dim
HPC = 4           # heads per core
NCORES = 8
SCALE = HD ** -0.5

F32 = mybir.dt.float32
BF16 = mybir.dt.bfloat16

QTILE = 512       # q-tile width (free dim of score matmuls)
KCH = 128         # k-chunk (partition dim of score tiles)
NQT = N // QTILE  # 4
DCH = D // 128    # 8 d_model chunks
NJUNK = 12        # PE warm-up matmuls (~5us at mid p-state)


def build_kernel():
    nc = bacc.Bacc("TRN2", target_bir_lowering=False, debug=False,
                   num_devices=NCORES)

    # inputs pre-tiled on host into partition-major layouts so each load is
    # one DMA with fat (multi-KB) per-partition contiguous descriptors
    xb0 = nc.dram_tensor("xb0", [128, DCH * QTILE], BF16,
                         kind="ExternalInput").ap()
    xrest = nc.dram_tensor("xrest", [128, DCH * 3 * QTILE], BF16,
                           kind="ExternalInput").ap()
    wqk = nc.dram_tensor("wqk", [128, 2 * DCH * 256], BF16,
                         kind="ExternalInput").ap()
    wvo = nc.dram_tensor("wvo", [128, DCH * 256 + 2 * D], BF16,
                         kind="ExternalInput").ap()
    bqz = nc.dram_tensor("bqz", [128, HPC], F32, kind="ExternalInput").ap()
    sclz = nc.dram_tensor("sclz", [128, HPC], F32, kind="ExternalInput").ap()
    bkd = nc.dram_tensor("bk", [HPC * HD], F32, kind="ExternalInput").ap()
    maskd = nc.dram_tensor("mask", [128, 896], BF16, kind="ExternalInput").ap()
    y = nc.dram_tensor("y", [N, D], BF16, kind="ExternalOutput").ap()

    Exp = mybir.ActivationFunctionType.Exp
    Identity = mybir.ActivationFunctionType.Identity
    Mult = mybir.AluOpType.mult
    Add = mybir.AluOpType.add

    def middim(ap2d, stride, n):
        """Insert a middle dim [stride, n] into a [p, w] AP."""
        return bass.AP(tensor=ap2d.tensor, offset=ap2d.offset,
                       ap=[list(ap2d.ap[0]), [stride, n], list(ap2d.ap[1])])

    with tile.TileContext(nc) as tc:
        from contextlib import ExitStack
        with ExitStack() as ctx:
            singles = ctx.enter_context(tc.tile_pool(name="singles", bufs=1))
            pt_pool = ctx.enter_context(tc.tile_pool(name="pt", bufs=4))
            r_pool = ctx.enter_context(tc.tile_pool(name="rp", bufs=2))
            yout = ctx.enter_context(tc.tile_pool(name="yout", bufs=3))
            dram = ctx.enter_context(
                tc.tile_pool(name="dram", bufs=2, space="DRAM"))
            ps_mm = ctx.enter_context(
                tc.tile_pool(name="psmm", bufs=2, space="PSUM"))
            ps_st = ctx.enter_context(
                tc.tile_pool(name="psst", bufs=2, space="PSUM"))
            ps_ot = ctx.enter_context(
                tc.tile_pool(name="psot", bufs=2, space="PSUM"))

            # PE warm-up scratch (memset first so the gpsimd queue is clean
            # and the junk matmuls can start the instant the PE is up)
            junk = singles.tile([128, 512], BF16)
            nc.gpsimd.memset(junk[:], 0.0)
            # ones rows for the 1/l PE-broadcast: partition 0 (odd heads)
            # and partition 64 (even heads)
            ones2 = singles.tile([65, 128], BF16, name="ones2")
            nc.gpsimd.memset(ones2[:], 1.0)
            ones1 = ones2[0:1, :]

            # --- resident inputs (DMA order = priority: first MMs need
            # wq/wk + x block 0; first pieces split small + spread across
            # queues so the Q projection can start ~12us in) ---------------
            wqk_r = wqk.rearrange("p (t o m) -> p t o m", t=2, o=DCH)
            wqk_sb = singles.tile([128, 2, DCH, 256], BF16)
            nc.sync.dma_start(wqk_sb[:, 0, :4], wqk_r[:, 0, :4])
            wq_sb = wqk_sb[:, 0]
            wk_sb = wqk_sb[:, 1]

            # x lives in one [128, k-chunk, token] tile; block 0 lands first
            # so compute can start early.
            XKA = singles.tile([128, DCH, N], BF16, name="xka")
            xk = [[XKA[:, k, b * QTILE:(b + 1) * QTILE] for b in range(NQT)]
                  for k in range(DCH)]
            xb0_r = xb0.rearrange("p (o m) -> p o m", o=DCH)
            nc.sync.dma_start(XKA[:, 0:4, 0:QTILE], xb0_r[:, 0:4])
            nc.sync.dma_start(wqk_sb[:, 0, 4:], wqk_r[:, 0, 4:])
            nc.sync.dma_start(XKA[:, 4:DCH, 0:QTILE], xb0_r[:, 4:])
            bqz_sb = singles.tile([128, HPC], F32)
            nc.gpsimd.dma_start(bqz_sb[:], bqz)
            sclz_sb = singles.tile([128, HPC], F32)
            nc.gpsimd.dma_start(sclz_sb[:], sclz)
            bk_sb = singles.tile([128, 2], F32)
            nc.gpsimd.dma_start(bk_sb[:], bkd.rearrange("(o p) -> p o", p=128))
            wvo_sb = singles.tile([128, DCH * 256 + 2 * D], BF16)
            wv_sb = wvo_sb[:, :DCH * 256].rearrange("p (o m) -> p o m", o=DCH)
            wop_sb = wvo_sb[:, DCH * 256:].rearrange("p (t m) -> p t m", t=2)
            xrest_r = xrest.rearrange("p (o b m) -> p o b m", o=DCH, b=3)
            xka_r = XKA[:, :, QTILE:].rearrange("p o (b m) -> p o b m", b=3)
            # wk on the (otherwise idle) scalar queue: transfers in
            # parallel with the sync-queue x pieces, so the K projection
            # isn't paced by a serial 2.5MB sync-queue window
            nc.scalar.dma_start(wqk_sb[:, 1], wqk_r[:, 1])

            # PE warm-up: the first ~8us are DMA-bound with the PE idle,
            # which leaves the PE clock throttled when real work starts.
            # Dependency-free junk matmuls unthrottle it.
            for i in range(NJUNK):
                wps = ps_ot.tile([128, 512], F32, tag="ot", name="wps")
                nc.tensor.matmul(wps[:], lhsT=junk[:, :128], rhs=junk[:],
                                 start=True, stop=True)

            mask_sb = singles.tile([128, 896], BF16)

            # V_aug layout per head (128 cols each inside V_sb):
            #   even h: cols 0-63 = V, col 64 = ones  -> l at partition 64
            #   odd h:  col 0 = ones, cols 64-127 = V -> l at partition 0
            QTz = [[singles.tile([128, QTILE], BF16, name=f"qtz{h}_{i}")
                    for i in range(NQT)] for h in range(HPC)]
            KT_sb = [singles.tile([128, 2, QTILE], BF16, name=f"kt{i}")
                     for i in range(NQT)]
            V_sb = [singles.tile([128, 4, HPC * 128], BF16, name=f"v{i}")
                    for i in range(NQT)]
            OTall = singles.tile([128, 2, N], BF16, name="otall")
            for blk in range(NQT):
                nc.gpsimd.memset(V_sb[blk][:], 0.0)
                for h in range(HPC):
                    oc = h * 128 + (64 if h % 2 == 0 else 0)
                    nc.gpsimd.memset(V_sb[blk][:, :, oc:oc + 1], 1.0)
                if blk == 0:
                    # second-priority loads: delayed by queue position until
                    # ~13us so the critical 2MB (wq/wk/x0) lands first, but
                    # still in time for project(0)'s V units / attend(0)
                    nc.gpsimd.dma_start(wvo_sb[:, :DCH * 256],
                                        wvo[:, :DCH * 256])
                    nc.gpsimd.dma_start(xka_r[:, :, 0], xrest_r[:, :, 0])
            # bulk transfers (x blocks 2-3, mask, Wo tail) dispatched from
            # the gpsimd queue AFTER the V memsets: a pure time delay that
            # keeps their ~2.7MB off the HBM while the critical head loads
            # (wq/wk/x0/x1/wv) are in flight -- no data-hazard guards needed
            nc.gpsimd.dma_start(mask_sb[:], maskd)
            nc.gpsimd.dma_start(xka_r[:, :, 1:], xrest_r[:, :, 1:])
            nc.gpsimd.dma_start(wvo_sb[:, DCH * 256:], wvo[:, DCH * 256:])

            # ---------------- project / outproj units --------------------
            def unit_q(blk, m):
                def emit():
                    ps = ps_mm.tile([128, 512], F32, tag="mm", name="psq")
                    for k in range(DCH):
                        nc.tensor.matmul(
                            ps[:],
                            lhsT=wq_sb[:, k, m * 128:(m + 1) * 128],
                            rhs=xk[k][blk][:],
                            start=(k == 0), stop=(k == DCH - 1))
                    for hh in range(2):
                        h = 2 * m + hh
                        # per-partition scale zeroes the other head's rows
                        nc.vector.tensor_scalar(
                            out=QTz[h][blk][:], in0=ps[:],
                            scalar1=sclz_sb[:, h:h + 1],
                            scalar2=bqz_sb[:, h:h + 1],
                            op0=Mult, op1=Add)
                return emit

            def unit_k(blk, m):
                def emit():
                    ps = ps_mm.tile([128, 512], F32, tag="mm", name="psk")
                    for k in range(DCH):
                        nc.tensor.matmul(
                            ps[:],
                            lhsT=wk_sb[:, k, m * 128:(m + 1) * 128],
                            rhs=xk[k][blk][:],
                            start=(k == 0), stop=(k == DCH - 1))
                    nc.vector.tensor_scalar_add(
                        out=KT_sb[blk][:, m, :], in0=ps[:],
                        scalar1=bk_sb[:, m:m + 1])
                return emit

            def unit_v(blk, tt):
                def emit():
                    ps = ps_mm.tile([128, 512], F32, tag="mm", name="psv")
                    for k in range(DCH):
                        nc.tensor.matmul(
                            ps[:, :HPC * HD],
                            lhsT=xk[k][blk][:, tt * 128:(tt + 1) * 128],
                            rhs=wv_sb[:, k, :],
                            start=(k == 0), stop=(k == DCH - 1))
                    # batched casts: even heads (psum 0-63,128-191 ->
                    # V cols 0-63,256-319), odd (64-127,192-255 ->
                    # 192-255,448-511)
                    nc.vector.tensor_copy(
                        middim(V_sb[blk][:, tt, 0:64], 256, 2),
                        middim(ps[:, 0:64], 128, 2))
                    nc.vector.tensor_copy(
                        middim(V_sb[blk][:, tt, 192:256], 256, 2),
                        middim(ps[:, 64:128], 128, 2))
                return emit

            def unit_o(qi, tt, act_ok=False):
                # output projection for one 128-token chunk (both halves).
                # When popped as an attend filler the drains stay off ACT so
                # they never delay the exp chain; at the tail ACT is idle
                # and takes one half.
                def emit():
                    t0 = qi * QTILE + tt * 128
                    y_sb = yout.tile([128, 1024], BF16, tag="y", name="yt")
                    for half in range(2):
                        ps = ps_mm.tile([128, 512], F32, tag="mm", name="psy")
                        for m in range(2):
                            nc.tensor.matmul(
                                ps[:],
                                lhsT=OTall[:, m, t0:t0 + 128],
                                rhs=wop_sb[:, m, half * 512:half * 512 + 512],
                                start=(m == 0), stop=(m == 1))
                        if half == 0 and act_ok:
                            nc.scalar.activation(
                                y_sb[:, :512], ps[:], Identity)
                        else:
                            nc.vector.tensor_copy(
                                y_sb[:, half * 512:half * 512 + 512], ps[:])
                    nc.sync.dma_start(y[t0:t0 + 128, :], y_sb[:])
                return emit

            def unit_junk(n=4):
                # dependency-free PE filler: keeps the clock at full p-state
                # through stretches where real PE work is dependency-gated
                def emit():
                    for _ in range(n):
                        wps = ps_ot.tile([128, 512], F32, tag="ot",
                                         name="wpsf")
                        nc.tensor.matmul(wps[:], lhsT=junk[:, :128],
                                         rhs=junk[:], start=True, stop=True)
                return emit

            # ---------------- attention ----------------------------------
            fillers = []

            def attend(qi, carry, heads, fill_stride, last=False):
                q0 = qi * QTILE
                npair_done = 0
                for hx, h in enumerate(heads):
                    mi = h // 2
                    po = (h % 2) * 64          # OT partition offset
                    vbase = h * 128
                    nrows = 65 if h % 2 == 0 else 128
                    lrow = 64 if h % 2 == 0 else 0
                    nprs = 2 * (qi + 1)        # pairs of 128-k-chunks
                    pso = ps_ot.tile([128, 512], F32, tag="ot", name="pso")

                    pair_ps = [None] * nprs
                    pair_pt = [None] * nprs

                    def consume(pi):
                        # last pair of each q-tile: only columns q >= r are
                        # unmasked (r = 256, 384); compute just those.
                        shrunk = (pi == nprs - 1)
                        pss, pt = pair_ps[pi], pair_pt[pi]
                        if shrunk:
                            for j in range(2):
                                r = (2 * pi + j) * KCH - q0
                                nc.scalar.activation(
                                    pt[:, j, r:], pss[:, j, r:], Exp)
                        else:
                            nc.scalar.activation(pt[:], pss[:], Exp)
                        for j in range(2):
                            ki = 2 * pi + j
                            r = ki * KCH - q0
                            if r >= 0:
                                # only the 128-col diagonal band is partially
                                # masked; the keep-pattern there is constant
                                nc.vector.tensor_mul(
                                    pt[:, j, r:r + 128], pt[:, j, r:r + 128],
                                    mask_sb[:, 384:512])
                        for j in range(2):
                            ki = 2 * pi + j
                            s = max(ki * KCH - q0, 0)
                            nc.tensor.matmul(
                                pso[:nrows, s:],
                                lhsT=V_sb[ki // 4][:, ki % 4,
                                                   vbase:vbase + nrows],
                                rhs=pt[:, j, s:],
                                start=(ki == 0), stop=(ki == 4 * (qi + 1) - 1))

                    for pi in range(nprs):
                        pss = ps_st.tile([128, 2, 512], F32, tag="st",
                                         name="pss")
                        pair_ps[pi] = pss
                        pair_pt[pi] = pt_pool.tile([128, 2, 512], BF16,
                                                   tag="pt", name="pt")
                        for j in range(2):
                            ki = 2 * pi + j
                            s = (ki * KCH - q0) if pi == nprs - 1 else 0
                            nc.tensor.matmul(
                                pss[:, j, s:],
                                lhsT=KT_sb[ki // 4][:, mi,
                                                    (ki % 4) * 128:
                                                    (ki % 4) * 128 + 128],
                                rhs=QTz[h][qi][:, s:],
                                start=True, stop=True)
                        if pi > 0:
                            consume(pi - 1)
                        npair_done += 1
                        if npair_done % fill_stride == 0 and fillers:
                            fillers.pop(0)[1]()
                    consume(nprs - 1)

                    # normalize, pipelined: drain now (stage OT+l off PSUM,
                    # 1/l bounce through DRAM), finish (mul on GpSimd) two
                    # heads later so no engine waits on the bounce round trip.
                    # reciprocal ops must START at partition 0.
                    if len(carry) == 2:
                        carry.pop(0)()
                    if last:
                        # final phase: the DMA bounce's ~5us latency would
                        # sit on the critical path before the final out-proj.
                        # Broadcast (1/)l with a K=1 ones-matmul on the PE,
                        # drain through ACT, finish-mul straight off PSUM on
                        # the DVE.  For odd heads (l at partition 0) recip
                        # first then broadcast; for even heads (l at
                        # partition 64, where reciprocal misbehaves)
                        # broadcast the raw l and recip the full broadcast.
                        rlb = r_pool.tile([128, 512], BF16, tag="rlb",
                                          name="rlb")
                        rb_ps = ps_mm.tile([128, 512], F32, tag="mm",
                                           name="rbps")
                        if h % 2 == 0:
                            rb = r_pool.tile([128, 512], F32, tag="rb",
                                             name="rb")
                            nc.vector.tensor_copy(rlb[64:65, :],
                                                  pso[64:65, :])
                            nc.tensor.matmul(rb_ps[:], lhsT=ones2[64:65, :],
                                             rhs=rlb[64:65, :], start=True,
                                             stop=True)
                            unit_junk(2)()
                            nc.vector.reciprocal_approx_fast(
                                out=rb[:], in_=rb_ps[:])
                            nc.vector.tensor_mul(
                                OTall[po:po + HD, mi, q0:q0 + QTILE],
                                pso[po:po + HD, :], rb[po:po + HD, :])
                        else:
                            rl = r_pool.tile([128, 512], F32, tag="rl",
                                             name="rl")
                            nc.vector.reciprocal_approx_fast(
                                out=rl[0:1, :], in_=pso[0:1, :])
                            nc.vector.tensor_copy(rlb[0:1, :], rl[0:1, :])
                            nc.tensor.matmul(rb_ps[:], lhsT=ones1[0:1, :],
                                             rhs=rlb[0:1, :], start=True,
                                             stop=True)
                            rb = r_pool.tile([128, 512], F32, tag="rb",
                                             name="rb")
                            nc.scalar.activation(rb[po:po + HD, :],
                                                 rb_ps[po:po + HD, :],
                                                 Identity)
                            unit_junk(5)()
                            nc.vector.tensor_mul(
                                OTall[po:po + HD, mi, q0:q0 + QTILE],
                                pso[po:po + HD, :], rb[po:po + HD, :])
                        continue
                    otu = r_pool.tile([128, 512], F32, tag="otu", name="otu")
                    nc.vector.tensor_copy(otu[:nrows, :], pso[:nrows, :])
                    sc = dram.tile([1, 512], F32, tag="sc", name="sc")
                    rb = r_pool.tile([128, 512], F32, tag="rb", name="rb")
                    if h % 2 == 0:
                        # l at partition 64: broadcast raw l, then recip the
                        # 64 partitions we need (offset 0).
                        nc.sync.dma_start(sc[:], otu[lrow:lrow + 1, :])
                        row = sc[0, :]
                        bcast = bass.AP(tensor=row.tensor, offset=row.offset,
                                        ap=[[0, HD]] + list(row.ap))
                        nc.sync.dma_start(rb[:HD, :], bcast)

                        def finish(otu=otu, rb=rb, po=po, mi=mi):
                            nc.vector.reciprocal_approx_fast(
                                out=rb[:HD, :], in_=rb[:HD, :])
                            nc.gpsimd.tensor_mul(
                                OTall[po:po + HD, mi, q0:q0 + QTILE],
                                otu[po:po + HD, :], rb[po:po + HD, :])
                    else:
                        # l at partition 0: recip the single row first, then
                        # broadcast 1/l.
                        rl = r_pool.tile([128, 512], F32, tag="rl", name="rl")
                        nc.vector.reciprocal_approx_fast(
                            out=rl[0:1, :], in_=otu[0:1, :])
                        nc.sync.dma_start(sc[:], rl[0:1, :])
                        row = sc[0, :]
                        bcast = bass.AP(tensor=row.tensor, offset=row.offset,
                                        ap=[[0, HD]] + list(row.ap))
                        nc.sync.dma_start(rb[HD:, :], bcast)

                        def finish(otu=otu, rb=rb, po=po, mi=mi):
                            nc.gpsimd.tensor_mul(
                                OTall[po:po + HD, mi, q0:q0 + QTILE],
                                otu[po:po + HD, :], rb[po:po + HD, :])
                    carry.append(finish)

            # ---------------- main schedule ------------------------------
            def proj_units(blk):
                return [unit_q(blk, 0), unit_q(blk, 1),
                        unit_k(blk, 0), unit_k(blk, 1)] + \
                       [unit_v(blk, tt) for tt in range(4)]

            def flush_proj_fillers():
                # projection units must precede the next attend phase;
                # outproj/junk units may carry over as fillers there
                keep = [f for f in fillers if f[0] != 'p']
                for kind, fn in fillers:
                    if kind == 'p':
                        fn()
                fillers[:] = keep

            def flush_fillers():
                while fillers:
                    fillers.pop(0)[1]()

            carry = []

            def flush_carry():
                while carry:
                    carry.pop(0)()

            for u in proj_units(0):
                u()
            fillers += [('p', u) for u in proj_units(1)]
            attend(0, carry, [0, 1, 2, 3], fill_stride=1)
            flush_proj_fillers()
            flush_carry()

            fillers += [('p', u) for u in proj_units(2)] + \
                       [('o', unit_o(0, tt)) for tt in range(4)]
            attend(1, carry, [0, 1, 2, 3], fill_stride=2)
            flush_proj_fillers()
            flush_carry()

            # project(3): K and V and Q(m=1) must precede attend(3) heads
            # 2,3; Q(m=0) only precedes heads 0,1 and fills part 1.
            fillers += [('p', u) for u in
                        [unit_k(3, 0), unit_k(3, 1)] +
                        [unit_v(3, tt) for tt in range(4)] +
                        [unit_q(3, 1)]] + \
                       [('o', unit_o(1, 0)), ('o', unit_o(1, 1))]
            attend(2, carry, [0, 1, 2, 3], fill_stride=3)
            flush_proj_fillers()
            flush_carry()

            fillers += [('p', unit_q(3, 0)),
                        ('o', unit_o(1, 2)), ('o', unit_o(1, 3)),
                        ('o', unit_o(2, 0)), ('o', unit_o(2, 1))]
            attend(3, carry, [2, 3], fill_stride=3)
            flush_proj_fillers()
            flush_carry()

            fillers += [('o', unit_o(2, 2)), ('o', unit_o(2, 3)),
                        ('o', unit_junk(3))]
            attend(3, carry, [0, 1], fill_stride=2, last=True)
            flush_carry()
            flush_fillers()
            for tt in range(4):
                unit_o(3, tt, act_ok=True)()

    nc.compile()
    return nc


def make_in_maps(x, Wq, bq, Wkv, bkv, Wo, bo):
    import ml_dtypes
    BF = ml_dtypes.bfloat16

    x = np.asarray(x, np.float32)
    Wq = np.asarray(Wq, np.float32)
    bq = np.asarray(bq, np.float32)
    Wkv = np.asarray(Wkv, np.float32)
    bkv = np.asarray(bkv, np.float32)
    Wo = np.asarray(Wo, np.float32)

    Wk, Wv = Wkv[:D], Wkv[D:]
    bk, bv = bkv[:D], bkv[D:]

    # mask[kk, u] = 1 iff u >= kk + 384 ; the kernel uses the [384:512]
    # slice (the diagonal band's keep-mask).
    u = np.arange(896)[None, :]
    kk = np.arange(128)[:, None]
    mask = (u >= kk + 384).astype(BF)

    in_maps = []
    for c in range(NCORES):
        b = c // (NCORES // B)
        hs = HPC * (c % (NCORES // B))
        rows = slice(hs * HD, hs * HD + HPC * HD)
        # SCALE folded into Wq/bq on host; sclz is a pure 0/1 row mask.
        bq_c = bq[rows] * SCALE
        bqz = np.zeros((128, HPC), np.float32)
        sclz = np.zeros((128, HPC), np.float32)
        for h in range(HPC):
            po = (h % 2) * 64
            m = h // 2
            bqz[po:po + 64, h] = bq_c[m * 128 + po:m * 128 + po + 64]
            sclz[po:po + 64, h] = 1.0
        # WoP: head pairs stacked per 128 partitions, [128, 2, 1024]
        wo_c = np.ascontiguousarray(Wo[:, rows].T)          # [256, 1024]
        woP = wo_c.reshape(2, 128, D).transpose(1, 0, 2)    # [128, 2, 1024]
        # partition-major packed loads: [p][...] contiguous per partition
        def pmaj(w):      # [1024, 256] -> [128, 8, 256]
            return w.reshape(DCH, 128, HPC * HD).transpose(1, 0, 2)
        wqk_h = np.concatenate(
            [pmaj(Wq[rows].T * SCALE)[:, None], pmaj(Wk[rows].T)[:, None]],
            axis=1).reshape(128, 2 * DCH * 256)
        wvo_h = np.concatenate(
            [pmaj(Wv[rows].T).reshape(128, DCH * 256),
             woP.reshape(128, 2 * D)], axis=1)
        xr = x[b].T.reshape(DCH, 128, N).transpose(1, 0, 2)  # [128, 8, 2048]
        in_maps.append({
            "xb0": np.ascontiguousarray(
                xr[:, :, :QTILE].reshape(128, DCH * QTILE)).astype(BF),
            "xrest": np.ascontiguousarray(
                xr[:, :, QTILE:].reshape(128, DCH * 3 * QTILE)).astype(BF),
            "wqk": np.ascontiguousarray(wqk_h).astype(BF),
            "wvo": np.ascontiguousarray(wvo_h).astype(BF),
            "bqz": bqz,
            "sclz": sclz,
            "bk": np.ascontiguousarray(bk[rows]),
            "mask": mask,
        })
    return in_maps


_NC_CACHE = None


def _get_nc():
    global _NC_CACHE
    if _NC_CACHE is None:
        _NC_CACHE = build_kernel()
    return _NC_CACHE


def kernel(x, Wq, bq, Wkv, bkv, Wo, bo, _trace=False, _trace_kwargs=None):
    nc = _get_nc()
    in_maps = make_in_maps(x, Wq, bq, Wkv, bkv, Wo, bo)
    kwargs = {}
    if _trace:
        kwargs = dict(trace=True, trace_cores=list(range(NCORES)),
                      **(_trace_kwargs or {}))
    res = run_bass_kernel_spmd(nc, in_maps, core_ids=list(range(NCORES)),
                               **kwargs)
    out = np.zeros((B, N, D), np.float32)
    for c, r in enumerate(res.results):
        out[c // (NCORES // B)] += np.asarray(r["y"], np.float32)
    bv = np.asarray(bkv, np.float32)[D:]
    Wo_f = np.asarray(Wo, np.float32)
    out += (np.asarray(bo, np.float32) + bv @ Wo_f.T)[None, None, :]
    if _trace:
        kernel.last_results = res
    return out


# revision 51
# speedup vs baseline: 1.0246x; 1.0246x over previous
"""Trainium2 Bass kernel for causal multi-head attention (b=2, n=2048, d=1024, h=16).

Sharding: 8 cores; core c handles batch (c // 4) and the 4 heads
[4*(c%4), 4*(c%4)+4).  Each core computes its heads' attention plus its
partial output projection y_part = O_heads @ Wo[:, cols].T ; the host sums
the 4 per-batch partials (bf16 on the wire) and adds bo + bv @ Wo.T (the
V-bias contribution commutes with the softmax average, so it never goes on
device).

All matmuls run in bf16 with fp32 PSUM accumulation.  The PE is the
bottleneck engine (~119us of streaming at 2.4 GHz), and TRN2 halves the PE
clock for 3us after any idle gap, so the emission schedule is built around
keeping the PE saturated:

  - engine assignment: ACT does (almost) nothing but the softmax exp;
    Q/K/V projection drains run on the DVE as tensor_scalar ops (the
    per-partition scale/bias replaces ACT's activation bias/scale);
    normalize finish-muls run on GpSimd (SBUF-only engine); y drains are
    split ACT/DVE.
  - projection and output-projection matmul "units" (one PSUM accumulation
    group each) are queued as fillers and emitted BETWEEN attention pairs,
    absorbing the ACT-vs-PE drift (exp takes ~1.14us per 1024-col pair vs
    ~0.85us of PE work per pair).  Projection units flush before the
    attend phase that needs them; outproj units carry across boundaries.
  - head: the first wq/x pieces are split small, and the bulk transfers
    (x blocks 2-3, Wo, mask) are gated behind an SBUF->SBUF guard DMA
    sourced from x block 0, so the critical first ~4MB gets the full HBM
    bandwidth while warm-up junk matmuls hold the PE clock at speed.
  - tail: the last q-tile processes heads in order (2,3,0,1), holds back
    outproj(1)/outproj(2) units as late fillers, and normalizes its final
    two heads with a PE ones-matmul broadcast of (1/)l instead of the DMA
    bounce, with a couple of junk matmuls keeping the clock hot through
    the dependency chain.

Causal masking multiplies only the 128-col diagonal band of each PT chunk
with a constant 0/1 mask slice; fully-masked columns are never exp'd or
streamed.  The softmax denominator l rides the PV matmul as a ones column
of V_aug (partition 64 for even heads / 0 for odd); 1/l is broadcast
across partitions with an SBUF->DRAM->SBUF DMA bounce, finished two heads
later so the engines never stall on the round trip.
"""

import numpy as np

import concourse.bass as bass
import concourse.mybir as mybir
import concourse.tile as tile
from concourse import bacc
from concourse.bass_utils import run_bass_kernel_spmd

D = 1024          # d_model
N = 2048          # sequence length
B = 2             # batch
H_TOT = 16        # total heads
HD = 64           # head dim
HPC = 4           # heads per core
NCORES = 8
SCALE = HD ** -0.5

F32 = mybir.dt.float32
BF16 = mybir.dt.bfloat16

QTILE = 512       # q-tile width (free dim of score matmuls)
KCH = 128         # k-chunk (partition dim of score tiles)
NQT = N // QTILE  # 4
DCH = D // 128    # 8 d_model chunks
NJUNK = 12        # PE warm-up matmuls (~5us at mid p-state)


def build_kernel():
    nc = bacc.Bacc("TRN2", target_bir_lowering=False, debug=False,
                   num_devices=NCORES)

    # inputs pre-tiled on host into partition-major layouts so each load is
    # one DMA with fat (multi-KB) per-partition contiguous descriptors
    xb0 = nc.dram_tensor("xb0", [128, DCH * QTILE], BF16,
                         kind="ExternalInput").ap()
    xrest = nc.dram_tensor("xrest", [128, DCH * 3 * QTILE], BF16,
                           kind="ExternalInput").ap()
    wqk = nc.dram_tensor("wqk", [128, 2 * DCH * 256], BF16,
                         kind="ExternalInput").ap()
    wvo = nc.dram_tensor("wvo", [128, DCH * 256 + 2 * D], BF16,
                         kind="ExternalInput").ap()
    bqz = nc.dram_tensor("bqz", [128, HPC], F32, kind="ExternalInput").ap()
    sclz = nc.dram_tensor("sclz", [128, HPC], F32, kind="ExternalInput").ap()
    bkd = nc.dram_tensor("bk", [HPC * HD], F32, kind="ExternalInput").ap()
    maskd = nc.dram_tensor("mask", [128, 896], BF16, kind="ExternalInput").ap()
    y = nc.dram_tensor("y", [N, D], BF16, kind="ExternalOutput").ap()

    Exp = mybir.ActivationFunctionType.Exp
    Identity = mybir.ActivationFunctionType.Identity
    Mult = mybir.AluOpType.mult
    Add = mybir.AluOpType.add

    def middim(ap2d, stride, n):
        """Insert a middle dim [stride, n] into a [p, w] AP."""
        return bass.AP(tensor=ap2d.tensor, offset=ap2d.offset,
                       ap=[list(ap2d.ap[0]), [stride, n], list(ap2d.ap[1])])

    with tile.TileContext(nc) as tc:
        from contextlib import ExitStack
        with ExitStack() as ctx:
            singles = ctx.enter_context(tc.tile_pool(name="singles", bufs=1))
            pt_pool = ctx.enter_context(tc.tile_pool(name="pt", bufs=6))
            r_pool = ctx.enter_context(tc.tile_pool(name="rp", bufs=3))
            yout = ctx.enter_context(tc.tile_pool(name="yout", bufs=3))
            dram = ctx.enter_context(
                tc.tile_pool(name="dram", bufs=4, space="DRAM"))
            ps_mm = ctx.enter_context(
                tc.tile_pool(name="psmm", bufs=2, space="PSUM"))
            ps_st = ctx.enter_context(
                tc.tile_pool(name="psst", bufs=2, space="PSUM"))
            ps_ot = ctx.enter_context(
                tc.tile_pool(name="psot", bufs=2, space="PSUM"))

            # PE warm-up scratch (memset first so the gpsimd queue is clean
            # and the junk matmuls can start the instant the PE is up)
            junk = singles.tile([128, 512], BF16)
            nc.gpsimd.memset(junk[:], 0.0)
            # ones rows for the 1/l PE-broadcast: partition 0 (odd heads)
            # and partition 64 (even heads)
            ones2 = singles.tile([65, 128], BF16, name="ones2")
            nc.gpsimd.memset(ones2[:], 1.0)
            ones1 = ones2[0:1, :]

            # --- resident inputs (DMA order = priority: first MMs need
            # wq/wk + x block 0; first pieces split small + spread across
            # queues so the Q projection can start ~12us in) ---------------
            wqk_r = wqk.rearrange("p (t o m) -> p t o m", t=2, o=DCH)
            wqk_sb = singles.tile([128, 2, DCH, 256], BF16)
            nc.sync.dma_start(wqk_sb[:, 0, :4], wqk_r[:, 0, :4])
            wq_sb = wqk_sb[:, 0]
            wk_sb = wqk_sb[:, 1]

            # x lives in one [128, k-chunk, token] tile; block 0 lands first
            # so compute can start early.
            XKA = singles.tile([128, DCH, N], BF16, name="xka")
            xk = [[XKA[:, k, b * QTILE:(b + 1) * QTILE] for b in range(NQT)]
                  for k in range(DCH)]
            xb0_r = xb0.rearrange("p (o m) -> p o m", o=DCH)
            nc.sync.dma_start(XKA[:, 0:4, 0:QTILE], xb0_r[:, 0:4])
            nc.sync.dma_start(wqk_sb[:, 0, 4:], wqk_r[:, 0, 4:])
            nc.sync.dma_start(XKA[:, 4:DCH, 0:QTILE], xb0_r[:, 4:])
            bqz_sb = singles.tile([128, HPC], F32)
            nc.gpsimd.dma_start(bqz_sb[:], bqz)
            sclz_sb = singles.tile([128, HPC], F32)
            nc.gpsimd.dma_start(sclz_sb[:], sclz)
            bk_sb = singles.tile([128, 2], F32)
            nc.gpsimd.dma_start(bk_sb[:], bkd.rearrange("(o p) -> p o", p=128))
            wvo_sb = singles.tile([128, DCH * 256 + 2 * D], BF16)
            wv_sb = wvo_sb[:, :DCH * 256].rearrange("p (o m) -> p o m", o=DCH)
            wop_sb = wvo_sb[:, DCH * 256:].rearrange("p (t m) -> p t m", t=2)
            xrest_r = xrest.rearrange("p (o b m) -> p o b m", o=DCH, b=3)
            xka_r = XKA[:, :, QTILE:].rearrange("p o (b m) -> p o b m", b=3)
            nc.sync.dma_start(wqk_sb[:, 1], wqk_r[:, 1])

            # PE warm-up: the first ~8us are DMA-bound with the PE idle,
            # which leaves the PE clock throttled when real work starts.
            # Dependency-free junk matmuls unthrottle it.
            for i in range(NJUNK):
                wps = ps_ot.tile([128, 512], F32, tag="ot", name="wps")
                nc.tensor.matmul(wps[:], lhsT=junk[:, :128], rhs=junk[:],
                                 start=True, stop=True)

            mask_sb = singles.tile([128, 896], BF16)

            # V_aug layout per head (128 cols each inside V_sb):
            #   even h: cols 0-63 = V, col 64 = ones  -> l at partition 64
            #   odd h:  col 0 = ones, cols 64-127 = V -> l at partition 0
            QTz = [[singles.tile([128, QTILE], BF16, name=f"qtz{h}_{i}")
                    for i in range(NQT)] for h in range(HPC)]
            KT_sb = [singles.tile([128, 2, QTILE], BF16, name=f"kt{i}")
                     for i in range(NQT)]
            V_sb = [singles.tile([128, 4, HPC * 128], BF16, name=f"v{i}")
                    for i in range(NQT)]
            OTall = singles.tile([128, 2, N], BF16, name="otall")
            for blk in range(NQT):
                nc.gpsimd.memset(V_sb[blk][:], 0.0)
                for h in range(HPC):
                    oc = h * 128 + (64 if h % 2 == 0 else 0)
                    nc.gpsimd.memset(V_sb[blk][:, :, oc:oc + 1], 1.0)
                if blk == 0:
                    # second-priority loads: delayed by queue position until
                    # ~13us so the critical 2MB (wq/wk/x0) lands first, but
                    # still in time for project(0)'s V units / attend(0)
                    nc.gpsimd.dma_start(wvo_sb[:, :DCH * 256],
                                        wvo[:, :DCH * 256])
                    nc.gpsimd.dma_start(xka_r[:, :, 0], xrest_r[:, :, 0])
            # bulk transfers (x blocks 2-3, mask, Wo tail) dispatched from
            # the gpsimd queue AFTER the V memsets: a pure time delay that
            # keeps their ~2.7MB off the HBM while the critical head loads
            # (wq/wk/x0/x1/wv) are in flight -- no data-hazard guards needed
            nc.gpsimd.dma_start(mask_sb[:], maskd)
            nc.gpsimd.dma_start(xka_r[:, :, 1:], xrest_r[:, :, 1:])
            nc.gpsimd.dma_start(wvo_sb[:, DCH * 256:], wvo[:, DCH * 256:])

            # ---------------- project / outproj units --------------------
            def unit_q(blk, m):
                def emit():
                    ps = ps_mm.tile([128, 512], F32, tag="mm", name="psq")
                    for k in range(DCH):
                        nc.tensor.matmul(
                            ps[:],
                            lhsT=wq_sb[:, k, m * 128:(m + 1) * 128],
                            rhs=xk[k][blk][:],
                            start=(k == 0), stop=(k == DCH - 1))
                    for hh in range(2):
                        h = 2 * m + hh
                        # per-partition scale zeroes the other head's rows
                        nc.vector.tensor_scalar(
                            out=QTz[h][blk][:], in0=ps[:],
                            scalar1=sclz_sb[:, h:h + 1],
                            scalar2=bqz_sb[:, h:h + 1],
                            op0=Mult, op1=Add)
                return emit

            def unit_k(blk, m):
                def emit():
                    ps = ps_mm.tile([128, 512], F32, tag="mm", name="psk")
                    for k in range(DCH):
                        nc.tensor.matmul(
                            ps[:],
                            lhsT=wk_sb[:, k, m * 128:(m + 1) * 128],
                            rhs=xk[k][blk][:],
                            start=(k == 0), stop=(k == DCH - 1))
                    nc.vector.tensor_scalar_add(
                        out=KT_sb[blk][:, m, :], in0=ps[:],
                        scalar1=bk_sb[:, m:m + 1])
                return emit

            def unit_v(blk, tt):
                def emit():
                    ps = ps_mm.tile([128, 512], F32, tag="mm", name="psv")
                    for k in range(DCH):
                        nc.tensor.matmul(
                            ps[:, :HPC * HD],
                            lhsT=xk[k][blk][:, tt * 128:(tt + 1) * 128],
                            rhs=wv_sb[:, k, :],
                            start=(k == 0), stop=(k == DCH - 1))
                    # batched casts: even heads (psum 0-63,128-191 ->
                    # V cols 0-63,256-319), odd (64-127,192-255 ->
                    # 192-255,448-511)
                    nc.vector.tensor_copy(
                        middim(V_sb[blk][:, tt, 0:64], 256, 2),
                        middim(ps[:, 0:64], 128, 2))
                    nc.vector.tensor_copy(
                        middim(V_sb[blk][:, tt, 192:256], 256, 2),
                        middim(ps[:, 64:128], 128, 2))
                return emit

            def unit_o(qi, tt, act_ok=False):
                # output projection for one 128-token chunk (both halves).
                # When popped as an attend filler the drains stay off ACT so
                # they never delay the exp chain; at the tail ACT is idle
                # and takes one half.
                def emit():
                    t0 = qi * QTILE + tt * 128
                    y_sb = yout.tile([128, 1024], BF16, tag="y", name="yt")
                    for half in range(2):
                        ps = ps_mm.tile([128, 512], F32, tag="mm", name="psy")
                        for m in range(2):
                            nc.tensor.matmul(
                                ps[:],
                                lhsT=OTall[:, m, t0:t0 + 128],
                                rhs=wop_sb[:, m, half * 512:half * 512 + 512],
                                start=(m == 0), stop=(m == 1))
                        if half == 0 and act_ok:
                            nc.scalar.activation(
                                y_sb[:, :512], ps[:], Identity)
                        else:
                            nc.vector.tensor_copy(
                                y_sb[:, half * 512:half * 512 + 512], ps[:])
                    nc.sync.dma_start(y[t0:t0 + 128, :], y_sb[:])
                return emit

            def unit_junk(n=4):
                # dependency-free PE filler: keeps the clock at full p-state
                # through stretches where real PE work is dependency-gated
                def emit():
                    for _ in range(n):
                        wps = ps_ot.tile([128, 512], F32, tag="ot",
                                         name="wpsf")
                        nc.tensor.matmul(wps[:], lhsT=junk[:, :128],
                                         rhs=junk[:], start=True, stop=True)
                return emit

            # ---------------- attention ----------------------------------
            fillers = []

            def attend(qi, carry, heads, fill_stride, last=False):
                q0 = qi * QTILE
                npair_done = 0
                for hx, h in enumerate(heads):
                    mi = h // 2
                    po = (h % 2) * 64          # OT partition offset
                    vbase = h * 128
                    nrows = 65 if h % 2 == 0 else 128
                    lrow = 64 if h % 2 == 0 else 0
                    nprs = 2 * (qi + 1)        # pairs of 128-k-chunks
                    pso = ps_ot.tile([128, 512], F32, tag="ot", name="pso")

                    pair_ps = [None] * nprs
                    pair_pt = [None] * nprs

                    def consume(pi):
                        # last pair of each q-tile: only columns q >= r are
                        # unmasked (r = 256, 384); compute just those.
                        shrunk = (pi == nprs - 1)
                        pss, pt = pair_ps[pi], pair_pt[pi]
                        if shrunk:
                            for j in range(2):
                                r = (2 * pi + j) * KCH - q0
                                nc.scalar.activation(
                                    pt[:, j, r:], pss[:, j, r:], Exp)
                        else:
                            nc.scalar.activation(pt[:], pss[:], Exp)
                        for j in range(2):
                            ki = 2 * pi + j
                            r = ki * KCH - q0
                            if r >= 0:
                                # only the 128-col diagonal band is partially
                                # masked; the keep-pattern there is constant
                                nc.vector.tensor_mul(
                                    pt[:, j, r:r + 128], pt[:, j, r:r + 128],
                                    mask_sb[:, 384:512])
                        for j in range(2):
                            ki = 2 * pi + j
                            s = max(ki * KCH - q0, 0)
                            nc.tensor.matmul(
                                pso[:nrows, s:],
                                lhsT=V_sb[ki // 4][:, ki % 4,
                                                   vbase:vbase + nrows],
                                rhs=pt[:, j, s:],
                                start=(ki == 0), stop=(ki == 4 * (qi + 1) - 1))

                    for pi in range(nprs):
                        pss = ps_st.tile([128, 2, 512], F32, tag="st",
                                         name="pss")
                        pair_ps[pi] = pss
                        pair_pt[pi] = pt_pool.tile([128, 2, 512], BF16,
                                                   tag="pt", name="pt")
                        for j in range(2):
                            ki = 2 * pi + j
                            s = (ki * KCH - q0) if pi == nprs - 1 else 0
                            nc.tensor.matmul(
                                pss[:, j, s:],
                                lhsT=KT_sb[ki // 4][:, mi,
                                                    (ki % 4) * 128:
                                                    (ki % 4) * 128 + 128],
                                rhs=QTz[h][qi][:, s:],
                                start=True, stop=True)
                        if pi > 0:
                            consume(pi - 1)
                        npair_done += 1
                        if npair_done % fill_stride == 0 and fillers:
                            fillers.pop(0)[1]()
                    consume(nprs - 1)

                    # normalize, pipelined: drain now (stage OT+l off PSUM,
                    # 1/l bounce through DRAM), finish (mul on GpSimd) two
                    # heads later so no engine waits on the bounce round trip.
                    # reciprocal ops must START at partition 0.
                    if len(carry) == 2:
                        carry.pop(0)()
                    if last:
                        # final phase: the DMA bounce's ~5us latency would
                        # sit on the critical path before the final out-proj.
                        # Broadcast (1/)l with a K=1 ones-matmul on the PE,
                        # drain through ACT, finish-mul straight off PSUM on
                        # the DVE.  For odd heads (l at partition 0) recip
                        # first then broadcast; for even heads (l at
                        # partition 64, where reciprocal misbehaves)
                        # broadcast the raw l and recip the full broadcast.
                        rlb = r_pool.tile([128, 512], BF16, tag="rlb",
                                          name="rlb")
                        rb_ps = ps_mm.tile([128, 512], F32, tag="mm",
                                           name="rbps")
                        if h % 2 == 0:
                            rb = r_pool.tile([128, 512], F32, tag="rb",
                                             name="rb")
                            nc.vector.tensor_copy(rlb[64:65, :],
                                                  pso[64:65, :])
                            nc.tensor.matmul(rb_ps[:], lhsT=ones2[64:65, :],
                                             rhs=rlb[64:65, :], start=True,
                                             stop=True)
                            unit_junk(2)()
                            nc.vector.reciprocal_approx_fast(
                                out=rb[:], in_=rb_ps[:])
                            nc.vector.tensor_mul(
                                OTall[po:po + HD, mi, q0:q0 + QTILE],
                                pso[po:po + HD, :], rb[po:po + HD, :])
                        else:
                            rl = r_pool.tile([128, 512], F32, tag="rl",
                                             name="rl")
                            nc.vector.reciprocal_approx_fast(
                                out=rl[0:1, :], in_=pso[0:1, :])
                            nc.vector.tensor_copy(rlb[0:1, :], rl[0:1, :])
                            nc.tensor.matmul(rb_ps[:], lhsT=ones1[0:1, :],
                                             rhs=rlb[0:1, :], start=True,
                                             stop=True)
                            rb = r_pool.tile([128, 512], F32, tag="rb",
                                             name="rb")
                            nc.scalar.activation(rb[po:po + HD, :],
                                                 rb_ps[po:po + HD, :],
                                                 Identity)
                            unit_junk(5)()
                            nc.vector.tensor_mul(
                                OTall[po:po + HD, mi, q0:q0 + QTILE],
                                pso[po:po + HD, :], rb[po:po + HD, :])
                        continue
                    otu = r_pool.tile([128, 512], F32, tag="otu", name="otu")
                    nc.vector.tensor_copy(otu[:nrows, :], pso[:nrows, :])
                    sc = dram.tile([1, 512], F32, tag="sc", name="sc")
                    rb = r_pool.tile([128, 512], F32, tag="rb", name="rb")
                    if h % 2 == 0:
                        # l at partition 64: broadcast raw l, then recip the
                        # 64 partitions we need (offset 0).
                        nc.sync.dma_start(sc[:], otu[lrow:lrow + 1, :])
                        row = sc[0, :]
                        bcast = bass.AP(tensor=row.tensor, offset=row.offset,
                                        ap=[[0, HD]] + list(row.ap))
                        nc.sync.dma_start(rb[:HD, :], bcast)

                        def finish(otu=otu, rb=rb, po=po, mi=mi):
                            nc.vector.reciprocal_approx_fast(
                                out=rb[:HD, :], in_=rb[:HD, :])
                            nc.gpsimd.tensor_mul(
                                OTall[po:po + HD, mi, q0:q0 + QTILE],
                                otu[po:po + HD, :], rb[po:po + HD, :])
                    else:
                        # l at partition 0: recip the single row first, then
                        # broadcast 1/l.
                        rl = r_pool.tile([128, 512], F32, tag="rl", name="rl")
                        nc.vector.reciprocal_approx_fast(
                            out=rl[0:1, :], in_=otu[0:1, :])
                        nc.sync.dma_start(sc[:], rl[0:1, :])
                        row = sc[0, :]
                        bcast = bass.AP(tensor=row.tensor, offset=row.offset,
                                        ap=[[0, HD]] + list(row.ap))
                        nc.sync.dma_start(rb[HD:, :], bcast)

                        def finish(otu=otu, rb=rb, po=po, mi=mi):
                            nc.gpsimd.tensor_mul(
                                OTall[po:po + HD, mi, q0:q0 + QTILE],
                                otu[po:po + HD, :], rb[po:po + HD, :])
                    carry.append(finish)

            # ---------------- main schedule ------------------------------
            def proj_units(blk):
                return [unit_q(blk, 0), unit_q(blk, 1),
                        unit_k(blk, 0), unit_k(blk, 1)] + \
                       [unit_v(blk, tt) for tt in range(4)]

            def flush_proj_fillers():
                # projection units must precede the next attend phase;
                # outproj/junk units may carry over as fillers there
                keep = [f for f in fillers if f[0] != 'p']
                for kind, fn in fillers:
                    if kind == 'p':
                        fn()
                fillers[:] = keep

            def flush_fillers():
                while fillers:
                    fillers.pop(0)[1]()

            carry = []

            def flush_carry():
                while carry:
                    carry.pop(0)()

            for u in proj_units(0):
                u()
            fillers += [('p', u) for u in proj_units(1)]
            attend(0, carry, [0, 1, 2, 3], fill_stride=1)
            flush_proj_fillers()
            flush_carry()

            fillers += [('p', u) for u in proj_units(2)] + \
                       [('o', unit_o(0, tt)) for tt in range(4)]
            attend(1, carry, [0, 1, 2, 3], fill_stride=2)
            flush_proj_fillers()
            flush_carry()

            # project(3): K and V and Q(m=1) must precede attend(3) heads
            # 2,3; Q(m=0) only precedes heads 0,1 and fills part 1.
            fillers += [('p', u) for u in
                        [unit_k(3, 0), unit_k(3, 1)] +
                        [unit_v(3, tt) for tt in range(4)] +
                        [unit_q(3, 1)]] + \
                       [('o', unit_o(1, 0)), ('o', unit_o(1, 1))]
            attend(2, carry, [0, 1, 2, 3], fill_stride=3)
            flush_proj_fillers()
            flush_carry()

            fillers += [('p', unit_q(3, 0)),
                        ('o', unit_o(1, 2)), ('o', unit_o(1, 3)),
                        ('o', unit_o(2, 0)), ('o', unit_o(2, 1))]
            attend(3, carry, [2, 3], fill_stride=3)
            flush_proj_fillers()
            flush_carry()

            fillers += [('o', unit_o(2, 2)), ('o', unit_o(2, 3)),
                        ('o', unit_junk(3))]
            attend(3, carry, [0, 1], fill_stride=2, last=True)
            flush_carry()
            flush_fillers()
            for tt in range(4):
                unit_o(3, tt, act_ok=True)()

    nc.compile()
    return nc


def make_in_maps(x, Wq, bq, Wkv, bkv, Wo, bo):
    import ml_dtypes
    BF = ml_dtypes.bfloat16

    x = np.asarray(x, np.float32)
    Wq = np.asarray(Wq, np.float32)
    bq = np.asarray(bq, np.float32)
    Wkv = np.asarray(Wkv, np.float32)
    bkv = np.asarray(bkv, np.float32)
    Wo = np.asarray(Wo, np.float32)

    Wk, Wv = Wkv[:D], Wkv[D:]
    bk, bv = bkv[:D], bkv[D:]

    # mask[kk, u] = 1 iff u >= kk + 384 ; the kernel uses the [384:512]
    # slice (the diagonal band's keep-mask).
    u = np.arange(896)[None, :]
    kk = np.arange(128)[:, None]
    mask = (u >= kk + 384).astype(BF)

    in_maps = []
    for c in range(NCORES):
        b = c // (NCORES // B)
        hs = HPC * (c % (NCORES // B))
        rows = slice(hs * HD, hs * HD + HPC * HD)
        # SCALE folded into Wq/bq on host; sclz is a pure 0/1 row mask.
        bq_c = bq[rows] * SCALE
        bqz = np.zeros((128, HPC), np.float32)
        sclz = np.zeros((128, HPC), np.float32)
        for h in range(HPC):
            po = (h % 2) * 64
            m = h // 2
            bqz[po:po + 64, h] = bq_c[m * 128 + po:m * 128 + po + 64]
            sclz[po:po + 64, h] = 1.0
        # WoP: head pairs stacked per 128 partitions, [128, 2, 1024]
        wo_c = np.ascontiguousarray(Wo[:, rows].T)          # [256, 1024]
        woP = wo_c.reshape(2, 128, D).transpose(1, 0, 2)    # [128, 2, 1024]
        # partition-major packed loads: [p][...] contiguous per partition
        def pmaj(w):      # [1024, 256] -> [128, 8, 256]
            return w.reshape(DCH, 128, HPC * HD).transpose(1, 0, 2)
        wqk_h = np.concatenate(
            [pmaj(Wq[rows].T * SCALE)[:, None], pmaj(Wk[rows].T)[:, None]],
            axis=1).reshape(128, 2 * DCH * 256)
        wvo_h = np.concatenate(
            [pmaj(Wv[rows].T).reshape(128, DCH * 256),
             woP.reshape(128, 2 * D)], axis=1)
        xr = x[b].T.reshape(DCH, 128, N).transpose(1, 0, 2)  # [128, 8, 2048]
        in_maps.append({
            "xb0": np.ascontiguousarray(
                xr[:, :, :QTILE].reshape(128, DCH * QTILE)).astype(BF),
            "xrest": np.ascontiguousarray(
                xr[:, :, QTILE:].reshape(128, DCH * 3 * QTILE)).astype(BF),
            "wqk": np.ascontiguousarray(wqk_h).astype(BF),
            "wvo": np.ascontiguousarray(wvo_h).astype(BF),
            "bqz": bqz,
            "sclz": sclz,
            "bk": np.ascontiguousarray(bk[rows]),
            "mask": mask,
        })
    return in_maps


_NC_CACHE = None


def _get_nc():
    global _NC_CACHE
    if _NC_CACHE is None:
        _NC_CACHE = build_kernel()
    return _NC_CACHE


def kernel(x, Wq, bq, Wkv, bkv, Wo, bo, _trace=False, _trace_kwargs=None):
    nc = _get_nc()
    in_maps = make_in_maps(x, Wq, bq, Wkv, bkv, Wo, bo)
    kwargs = {}
    if _trace:
        kwargs = dict(trace=True, trace_cores=list(range(NCORES)),
                      **(_trace_kwargs or {}))
    res = run_bass_kernel_spmd(nc, in_maps, core_ids=list(range(NCORES)),
                               **kwargs)
    out = np.zeros((B, N, D), np.float32)
    for c, r in enumerate(res.results):
        out[c // (NCORES // B)] += np.asarray(r["y"], np.float32)
    bv = np.asarray(bkv, np.float32)[D:]
    Wo_f = np.asarray(Wo, np.float32)
    out += (np.asarray(bo, np.float32) + bv @ Wo_f.T)[None, None, :]
    if _trace:
        kernel.last_results = res
    return out
